# revision 51
# baseline (speedup 1.0000x reference)
"""Trainium2 Bass kernel for nn_FComb_79319456023150 (dense_cnn).

Per-pixel MLP over a 96^3 volume: four 1x1x1 convs (38->32->32->32->1 channels
with relu between). z is batch-constant, so w1[:, 32:38] @ z folds into the
layer-1 bias and every layer becomes a K=32 channel GEMM.

Sharding: spatial (outermost X axis) across 8 cores, 110592 pixels each.
Weights/biases replicated.

Device layout per core: the host restripes each shard to [128, 27648] = 4
pixel-blocks x 32 channels on partitions, pixels on the free dim, in bf16
(rel err ~5e-3, well under the 2e-2 gate; halves DMA vs fp32). Each layer is
computed with a BLOCK-DIAGONAL [128, 128] weight (4 copies of W^T on the
diagonal), so one full-array bf16 matmul per 512-col super-chunk applies the
32x32 GEMM to all 4 pixel blocks at once (1 col/cycle).

Pipeline: SEVEN super-chunk streams, one PSUM bank each; bank 7 is a
dedicated L4 accumulator. Relu+bias rides the mandatory PSUM->SBUF crossing,
which only Act and DVE may perform (GPSIMD cannot access PSUM), and their
combined throughput barely covers the demand - so crossings are COHORT ops
spanning adjacent streams' banks (contiguous PSUM): [128, 1024] over a
stream pair amortizes the per-op init (Act 185ns / DVE 125ns) that a
512-wide op can't afford. Each group's crossing-free L4 row is SOFTWARE
PIPELINED into the next group's layer-1 slot, so Act/DVE never idle through
a group boundary (-8us vs emitting it in place).

L4: each sc's matmul accumulates into bank 7 at rows 4j+m (j = sc index
within a 28-sc half), so HALF THE KERNEL's output evacuates with ONE
[112, 512] op (engines charge free size only) and ships as bf16 (host
converts back to fp32; +2e-4 rel err) with one DMA whose (j, m, n)
permutation lives entirely on the DRAM side - SBUF-side APs with two
partition dims mis-lower in walrus DMA codegen (hardware-verified failure
mode).
"""

import sys

import numpy as np

if "/opt/trn_rl_repo" not in sys.path:
    sys.path.insert(0, "/opt/trn_rl_repo")

C = 32          # channels per layer
P = 128         # SBUF/PSUM partitions
RG = 4          # pixel blocks stacked on the partition dim (128/32)
CH = 512        # super-chunk width = one PSUM bank of fp32
VOL = 96 * 96 * 96                   # full volume
NCORES = 8
NPIX = VOL // NCORES                 # 110592 pixels per core
FREE = NPIX // RG                    # 27648 free-dim columns per core
NSC = FREE // CH                     # 54 super-chunks per core
NS = 7                               # parallel sc streams (PSUM banks 0-6)
L4SPAN = 28                          # scs accumulated per L4-bank fill
assert FREE % CH == 0


# Crossing cohorts: layer-l results of streams [0,1], [2,3], [4,5] evacuate
# as [128,1024] ops; stream 6 as [128,512]. "pat" assigns engines per
# (group-parity, layer row) to the 4 cohort ops; Act is faster per column
# (1.2 vs 0.96 GHz) so it leans on the wide ops.
DEFAULT_CFG = {
    "pat": [
        [("act", "dve", "act", "dve"), ("dve", "act", "dve", "act"),
         ("act", "dve", "act", "dve")],
    ],
    "fin": "act",
    "g0split": [3, 4],
    "cohorts": [(0, 2), (2, 2), (4, 2), (6, 1)],
}


def _build_nc(npix=NPIX, cfg=None):
    import concourse.mybir as mybir
    from concourse import bacc
    from concourse.tile import TileContext

    f32 = mybir.dt.float32
    bf16 = mybir.dt.bfloat16
    Alu = mybir.AluOpType
    Act = mybir.ActivationFunctionType

    if cfg is None:
        cfg = DEFAULT_CFG
    pat = cfg["pat"]
    fin_cfg = cfg.get("fin", "act")
    cohorts = cfg.get("cohorts", [(0, 2), (2, 2), (4, 2), (6, 1)])
    rowspec = cfg.get("rowspec")

    free = npix // RG
    nsc = free // CH
    assert free % CH == 0 and nsc >= 1
    l4span = min(L4SPAN, nsc)

    nc = bacc.Bacc()
    fm = nc.dram_tensor("fm", [P, free], bf16, kind="ExternalInput")
    wst = nc.dram_tensor(
        "wst", [P, (3 + l4span) * P], bf16, kind="ExternalInput")
    bias = nc.dram_tensor("bias", [P, 4], f32, kind="ExternalInput")
    out = nc.dram_tensor("out", [npix], bf16, kind="ExternalOutput")

    # out[m*free + s*CH + n] viewed [m, s, n] for per-half stores
    out_r = out.rearrange("(m s n) -> m s n", m=RG, s=nsc, n=CH)

    sgroups = [list(range(g, min(g + NS, nsc)))
               for g in range(0, nsc, NS)]
    ts = cfg.get("tailsplit")
    if ts and len(sgroups[-1]) > max(ts):
        last = sgroups.pop()
        i = 0
        for w in ts:
            sgroups.append(last[i:i + w])
            i += w
        assert i == len(last), (ts, len(last))

    with TileContext(nc) as tc:
        with (
            tc.tile_pool(name="const", bufs=1) as constp,
            tc.tile_pool(name="data", bufs=cfg.get("xbufs", 4)) as datap,
            tc.tile_pool(name="acts", bufs=cfg.get("hbufs", 2)) as actp,
            tc.tile_pool(name="outs", bufs=2) as outsp,
            tc.tile_pool(name="psb", bufs=1, space="PSUM") as psb,
        ):
            # Startup critical path: the first matmul needs only the three
            # layer weights + the first data chunk; bias and the L4 weights
            # ride behind the first data batches.
            wtile = constp.tile([P, (3 + l4span) * P], bf16)
            btile = constp.tile([P, 4], f32)
            if cfg.get("w1first"):
                nc.sync.dma_start(wtile[:, :P], wst[:, :P])
            else:
                nc.sync.dma_start(wtile[:, :3 * P], wst[:, :3 * P])

            psum = psb.tile([P, 8 * CH], f32)
            l4ps = psum[:, NS * CH:(NS + 1) * CH]

            def xop_on(eng, out_ap, in_ap, bcol, relu):
                if eng == "act":
                    return nc.scalar.activation(
                        out_ap, in_ap, Act.Relu if relu else Act.Identity,
                        bias=bcol, scale=1.0,
                    )
                e = nc.vector if eng == "dve" else nc.gpsimd
                if relu:
                    return e.tensor_scalar(out_ap, in_ap, bcol, 0.0,
                                           Alu.add, Alu.max)
                return e.tensor_scalar(out_ap, in_ap, bcol, None, Alu.add)

            hcur = {}
            hl4 = {}
            pending_mm4 = []
            for gi, scs in enumerate(sgroups):
                # input DMA: the first group loads in pieces so early
                # streams start while later ones transfer; later groups as
                # one batched DMA each.
                if scs[0] == 0:
                    base = 0
                    for bi, blen in enumerate(cfg.get("g0split", [4, 3])):
                        xt = datap.tile([P, blen * CH], bf16, tag="x")
                        nc.sync.dma_start(
                            xt, fm[:, base * CH:(base + blen) * CH])
                        for i in range(blen):
                            hcur[base + i] = xt[:, i * CH:(i + 1) * CH]
                        base += blen
                        if bi == 0:
                            if cfg.get("w1first"):
                                nc.sync.dma_start(
                                    wtile[:, P:3 * P], wst[:, P:3 * P])
                            nc.sync.dma_start(btile, bias[:, :])
                        if bi == 1 or (bi == 0 and base == len(scs)):
                            nc.sync.dma_start(
                                wtile[:, 3 * P:], wst[:, 3 * P:])
                    assert base == len(scs)
                else:
                    xt = datap.tile([P, len(scs) * CH], bf16, tag="x")
                    nc.sync.dma_start(
                        xt, fm[:, scs[0] * CH:(scs[0] + len(scs)) * CH])
                    for i, s in enumerate(scs):
                        hcur[s] = xt[:, i * CH:(i + 1) * CH]

                def emit_mm4(mm4_scs):
                    # layer 4: sc s accumulates into the dedicated bank at
                    # rows 4*(s % l4span) + m; each l4span-half evacuates
                    # with ONE [4*l4span, 512] op + one output DMA.
                    for s in mm4_scs:
                        jj = s % l4span
                        mm4 = nc.tensor.matmul(
                            l4ps, wtile[:, (3 + jj) * P:(4 + jj) * P],
                            hl4[s],
                            start=(jj == 0), stop=(jj == l4span - 1
                                                   or s == nsc - 1),
                        )
                        NAME_INFO[mm4.ins.name] = (s, "mm4")
                        if jj == l4span - 1 or s == nsc - 1:
                            hbase = s - jj
                            nrow = RG * l4span
                            ob = outsp.tile([RG * l4span, CH], bf16,
                                            tag="ob")
                            fin = xop_on(fin_cfg, ob[:nrow, :],
                                         l4ps[:nrow, :],
                                         btile[:nrow, 3:4], relu=False)
                            NAME_INFO[fin.ins.name] = (
                                hbase, f"final.{fin_cfg}")
                            # ONE DMA per half: the SBUF side stays a
                            # plain single-partition-dim [4*(jj+1), 512]
                            # (2-partition-dim SBUF APs mis-lower in walrus
                            # DMA codegen); the permutation lives on the
                            # DRAM side as nested strides (j, m, n).
                            dmao = nc.sync.dma_start(
                                out_r[:, hbase:s + 1, :].rearrange(
                                    "m k n -> k m n"),
                                ob[:RG * (jj + 1), :],
                            )
                            NAME_INFO[dmao.ins.name] = (hbase, "dma_out")

                rowpat = pat[gi % len(pat)]
                for layer in range(3):
                    for j, s in enumerate(scs):
                        ps = psum[:, j * CH:(j + 1) * CH]
                        mm = nc.tensor.matmul(
                            ps, wtile[:, layer * P:(layer + 1) * P],
                            hcur[s], start=True, stop=True,
                        )
                        NAME_INFO[mm.ins.name] = (s, f"mm{layer}")
                    if layer == cfg.get('mm4slot', 1) and pending_mm4:
                        # previous group's L4 row rides here, so the
                        # engines' relu work stays contiguous across the
                        # group boundary instead of idling through two
                        # crossing-free PE rows.
                        emit_mm4(pending_mm4)
                        pending_mm4 = []
                    bcol = btile[:, layer:layer + 1]
                    if rowspec is not None:
                        row = rowspec[gi % len(rowspec)][layer]
                    else:
                        row = [(c0, clen, rowpat[layer][ci])
                               for ci, (c0, clen) in enumerate(cohorts)]
                    for ci, (c0, clen, eng) in enumerate(row):
                        if c0 >= len(scs):
                            continue
                        cl = min(clen, len(scs) - c0)
                        ps = psum[:, c0 * CH:(c0 + cl) * CH]
                        hn = actp.tile([P, cl * CH], bf16,
                                       tag=f"h{layer}c{ci}")
                        xop = xop_on(eng, hn[:, :], ps, bcol, relu=True)
                        NAME_INFO[xop.ins.name] = (
                            scs[c0], f"relu{layer}.c{ci}.{eng}")
                        for i in range(cl):
                            hcur[scs[c0 + i]] = hn[:, i * CH:(i + 1) * CH]

                for s in scs:
                    hl4[s] = hcur[s]
                pending_mm4 = pending_mm4 + list(scs)

            if pending_mm4:
                emit_mm4(pending_mm4)

    nc.compile()
    return nc


def _blockdiag4(wT):
    """[32, 32] -> [128, 128] block-diagonal with 4 copies."""
    out = np.zeros((P, P), dtype=np.float32)
    for b in range(RG):
        out[32 * b:32 * b + 32, 32 * b:32 * b + 32] = wT
    return out


def _prep_host_inputs(z, w1, b1, w2, b2, w3, b3, wl, bl):
    """Fold z into the layer-1 bias and build the device weight layouts."""
    import ml_dtypes

    f32 = np.float32
    b1e = (b1 + w1[:, C:] @ z[0]).astype(f32)          # [32]

    # w4 block j: L4-bank row 4j+m <- wl . (pixel-block m of half-member
    # j's sc) - j-major, matching the output DMA's (j, m, n) iteration.
    l4span = min(L4SPAN, NSC)
    w4 = np.zeros((P, l4span * P), dtype=f32)
    for j in range(l4span):
        for m in range(RG):
            w4[32 * m:32 * m + 32, j * P + RG * j + m] = wl[0, :]

    wst = np.concatenate(
        [
            _blockdiag4(w1[:, :C].T),
            _blockdiag4(w2.T),
            _blockdiag4(w3.T),
            w4,
        ],
        axis=1,
    ).astype(ml_dtypes.bfloat16)                        # [128, (3+28)*128]

    bias = np.zeros((P, 4), dtype=f32)
    bias[:, 0] = np.tile(b1e, RG)
    bias[:, 1] = np.tile(b2.astype(f32), RG)
    bias[:, 2] = np.tile(b3.astype(f32), RG)
    bias[:, 3] = f32(bl[0])
    return wst, bias


def _restripe(shard):
    """[32, npix] channel-major shard -> [128, npix/4] (block, channel) rows."""
    npix = shard.shape[1]
    return np.ascontiguousarray(
        shard.reshape(C, RG, npix // RG).transpose(1, 0, 2).reshape(P, npix // RG)
    )


_NC_CACHE = {}
NAME_INFO = {}   # instruction name -> (sc, stage) for profiling


def _run(feature_map, z, w1, b1, w2, b2, w3, b3, wl, bl, **spmd_kwargs):
    import ml_dtypes
    from concourse.bass_utils import run_bass_kernel_spmd

    feature_map = np.asarray(feature_map, dtype=np.float32)
    z = np.asarray(z, dtype=np.float32)
    w1, b1 = np.asarray(w1, np.float32), np.asarray(b1, np.float32)
    w2, b2 = np.asarray(w2, np.float32), np.asarray(b2, np.float32)
    w3, b3 = np.asarray(w3, np.float32), np.asarray(b3, np.float32)
    wl, bl = np.asarray(wl, np.float32), np.asarray(bl, np.float32)

    wst, bias = _prep_host_inputs(z, w1, b1, w2, b2, w3, b3, wl, bl)

    fm_flat = feature_map.reshape(C, VOL)
    in_maps = []
    for k in range(NCORES):
        shard = _restripe(fm_flat[:, k * NPIX:(k + 1) * NPIX]).astype(
            ml_dtypes.bfloat16
        )
        in_maps.append({"fm": shard, "wst": wst, "bias": bias})

    if "nc" not in _NC_CACHE:
        _NC_CACHE["nc"] = _build_nc()
    nc = _NC_CACHE["nc"]

    res = run_bass_kernel_spmd(nc, in_maps, core_ids=list(range(NCORES)), **spmd_kwargs)
    out = np.empty((VOL,), dtype=np.float32)
    for k in range(NCORES):
        out[k * NPIX:(k + 1) * NPIX] = np.asarray(
            res.results[k]["out"]).astype(np.float32)
    return out.reshape(1, 1, 96, 96, 96), res


def kernel(feature_map, z, w1, b1, w2, b2, w3, b3, wl, bl):
    out, _ = _run(feature_map, z, w1, b1, w2, b2, w3, b3, wl, bl)
    return out
